# revision 58
# baseline (speedup 1.0000x reference)
"""3-layer GCN (PyG gcn_norm semantics) on 8 Trainium2 NeuronCores.

Sharding: nodes are range-partitioned across the 8 cores (graph parallel).
Each core owns rows [c*S, (c+1)*S) of every activation and of the
segment-sum output.  The small weight matrices are replicated.  Per layer:

  1. the full node-feature table x is uploaded (fp8, core-major padded
     rows) to every core by input staging, so layer 1 needs neither a
     dense transform pass nor an AllGather: its gathers read raw x rows
     and W1 is applied after the segment-sum (linearity of the GCN
     aggregation).  Layers 2-3 read tables produced by the previous
     layer's chunk epilogues, AllGathered core-major by a single
     one-shot AllGather into a Shared DRAM tensor (single writer ->
     direct remote writes, ~2.5x the bandwidth of piecewise RDH
     AllGathers),
  2. each core processes the edges whose *destination* lands in its range:
     an indirect DMA per 32-block edge batch gathers table rows (pad
     slots gather valid row 0 and are nulled by zero one-hot columns), a
     norm-weighted one-hot block onehot[e, w] = (dstlocal[e] == w) * norm[e]
     resident in SBUF (loaded once at startup, reused all layers)
     supplies the scatter pattern, and fp8 PE matmuls accumulate
     psum[feat, w] += gathered[e, feat].T @ onehot[e, w]
     into the PSUM tile of the current destination chunk pair (edges are
     pre-sorted by destination chunk, then by source row for DMA
     locality),
  3. chunk epilogues run per chunk PAIR (one PSUM tile holds two
     adjacent 64-row chunks; the even chunk's accumulation start clears
     the whole zero region): one ReLU/bias (layer 1: deferred-W1 matmul
     first), one next-layer weight matmul, one cast and one shard write
     per 128 destination rows.

The gather tables travel in fp8e4; all accumulation (PSUM), the
epilogue, weights, and the output stay fp32/fp16.  The final layer
computes logits per chunk and a batched log_softmax at the end.

Host-side work is limited to sharding/index preprocessing: edge
partitioning by dst range, sort by (dst chunk, src row), degree counting,
normalization coefficients, and packing block-padded index/coefficient
arrays.  All dense float math (matmuls, gathers, segment sums,
activations, log_softmax) runs on the NeuronCores.
"""

import os

import numpy as np

import concourse.bass as bass
import concourse.bacc as bacc
import concourse.mybir as mybir
import concourse.tile as tile
from concourse.bass import ts
from concourse.bass_utils import run_bass_kernel_spmd

F32 = mybir.dt.float32
F16 = mybir.dt.float16
I32 = mybir.dt.int32
P = 128  # partition dim == feature dim

LAST_RESULT = None


class Cfg:
    def __init__(self, n_cores, n_nodes, n_class, gather_k,
                 msg_dtype=mybir.dt.float8e4, cw=64):
        assert n_nodes % n_cores == 0
        self.n_cores = n_cores
        self.n_nodes = n_nodes
        self.n_class = n_class
        self.gather_k = gather_k
        self.cw = cw                         # scatter window (chunk) width
        self.S = n_nodes // n_cores          # rows per core
        self.CH = -(-self.S // cw)           # cw-row chunks per core
        assert self.CH % 2 == 0
        self.S_pad = self.CH * cw
        assert self.S_pad % P == 0
        self.pad = self.S_pad - self.S
        self.T_pad = n_cores * self.S_pad    # gather-table rows
        self.msg_dtype = msg_dtype

    @property
    def np_msg(self):
        return np.dtype(mybir.dt.np(self.msg_dtype))


FULL = Cfg(n_cores=8, n_nodes=100000, n_class=10, gather_k=32)


def _preprocess(cfg, edge_index):
    """Shard + sort edges by destination, build block-padded device arrays.

    Returns (NB, blocks, per_core) where blocks[b] = (chunk_id, first, last)
    and per_core is a list of dicts with idx/meta arrays per core.
    """
    S, CH, K = cfg.S, cfg.CH, cfg.gather_k
    n = cfg.n_nodes
    src = np.concatenate([edge_index[0], np.arange(n, dtype=np.int64)])
    dst = np.concatenate([edge_index[1], np.arange(n, dtype=np.int64)])
    deg = np.bincount(dst, minlength=n).astype(np.float64)
    dis = 1.0 / np.sqrt(deg)
    val = (dis[src] * dis[dst]).astype(np.float32)

    core = dst // S
    # core-major table rows
    score = src // S
    srcp = (score * cfg.S_pad + (src - score * S)).astype(np.int64)
    dloc = dst - core * S
    chunk = dloc // cfg.cw
    w = (dloc % cfg.cw).astype(np.float32)

    G = cfg.n_cores * CH
    cc = (core * CH + chunk).astype(np.int64)
    counts = np.bincount(cc, minlength=G).reshape(cfg.n_cores, CH)
    Bc = np.maximum(1, -(-counts.max(axis=0) // P)).astype(np.int64)  # [CH]
    if os.environ.get("GCN_DR", "0") == "1":
        Bc += Bc % 2  # even per chunk so DoubleRow pairs stay chunk-pure
    NB = int(Bc.sum())
    Bc[-1] += (-NB) % K
    NB = int(Bc.sum())

    chunk_off = np.zeros(CH, np.int64)
    chunk_off[1:] = np.cumsum(Bc * P)[:-1]
    L = NB * P

    # slot assignment: edges of (core, chunk) go to chunk_off[chunk] + rank;
    # within a chunk, edges are ordered by source row so the indirect
    # gather walks the table monotonically (DRAM locality)
    order = np.lexsort((srcp, cc))
    scc = cc[order]
    gstart = np.searchsorted(scc, np.arange(G))
    ranks = np.arange(len(order)) - gstart[scc]
    slots = chunk_off[scc % CH] + ranks
    cores_sorted = scc // CH

    md = cfg.np_msg
    # pad slots gather row 0 (valid); their one-hot columns are zero so
    # the value never contributes
    idx_a = np.zeros((cfg.n_cores, L), np.int32)
    val_a = np.zeros((cfg.n_cores, L), np.float32)
    w_a = np.full((cfg.n_cores, L), -1, np.int64)
    idx_a[cores_sorted, slots] = srcp[order]
    val_a[cores_sorted, slots] = val[order]
    w_a[cores_sorted, slots] = w[order].astype(np.int64)

    blocks = []
    for c in range(CH):
        nb = int(Bc[c])
        for i in range(nb):
            blocks.append((c, i == 0, i == nb - 1))
    assert len(blocks) == NB

    per_core = []
    ar = np.arange(L)
    for c in range(cfg.n_cores):
        # one-hot scatter blocks: oh[e, w] = (w == wloc[e]) * val[e],
        # laid out for the device as [128 partitions(e), NB*cw(b, w)]
        oh = np.zeros((L, cfg.cw), md)
        m = w_a[c] >= 0
        oh[ar[m], w_a[c][m]] = val_a[c][m].astype(md)
        oh = np.ascontiguousarray(
            oh.reshape(NB, P, cfg.cw).transpose(1, 0, 2).reshape(P, NB * cfg.cw)
        )
        # device tile layout: idx[p, b] = src row of edge slot b*128+p
        per_core.append({
            "eidx": np.ascontiguousarray(idx_a[c].reshape(NB, P).T),
            "eoh": oh,
        })
    return NB, blocks, per_core


def _build_program(cfg, NB, blocks):
    nc = bacc.Bacc(
        "TRN2", target_bir_lowering=False, debug=False, num_devices=cfg.n_cores
    )
    CH, K, NC = cfg.CH, cfg.gather_k, cfg.n_class
    CW = cfg.cw
    MD = cfg.msg_dtype
    NG = NB // K
    rg = [list(range(cfg.n_cores))]
    use_dr = os.environ.get("GCN_DR", "0") == "1"
    DR = mybir.MatmulPerfMode.DoubleRow if use_dr else None

    # kernel I/O.  xf is the full (pre-AllGathered by input staging) node
    # feature table in fp8, core-major padded rows -- layer 1 gathers raw
    # x rows from it and applies W1 after the segment-sum (linearity),
    # so layer 1 needs neither a dense transform pass nor an AllGather.
    xf_in = nc.dram_tensor("xf", [cfg.T_pad, P], MD, kind="ExternalInput")
    eidx_in = nc.dram_tensor("eidx", [P, NB], I32, kind="ExternalInput")
    eoh_in = nc.dram_tensor("eoh", [P, NB * CW], MD, kind="ExternalInput")
    W_in = [
        nc.dram_tensor(f"W{i + 1}", [P, P], F16, kind="ExternalInput")
        for i in range(3)
    ]
    Wl_in = nc.dram_tensor("Wl", [P, NC], F16, kind="ExternalInput")
    b_in = [
        nc.dram_tensor(f"b{i + 1}", [P, 1], F32, kind="ExternalInput")
        for i in range(3)
    ]
    blT_in = nc.dram_tensor("blT", [P, NC], F32, kind="ExternalInput")
    out_t = nc.dram_tensor("logits", [CW, CH * NC], F32, kind="ExternalOutput")

    with tile.TileContext(nc) as tc:
        with (
            tc.tile_pool(name="const", bufs=1) as constp,
            tc.tile_pool(name="persist", bufs=1) as persist,
            tc.tile_pool(name="gather", bufs=6) as gatherp,
            tc.tile_pool(name="epi", bufs=3) as epip,
            tc.tile_pool(name="mpsum", bufs=3, space="PSUM") as mpsump,
            tc.tile_pool(name="opsum", bufs=4, space="PSUM") as opsump,
            tc.tile_pool(name="dram", bufs=1, space="DRAM") as dramp,
        ):
            W_t = []
            for i in range(3):
                wt = constp.tile([P, P], F16, name=f"w{i}")
                nc.sync.dma_start(wt[:], W_in[i][:])
                W_t.append(wt)
            Wl_t = constp.tile([P, NC], F16)
            nc.sync.dma_start(Wl_t[:], Wl_in[:])
            b_t = []
            for i in range(3):
                bt = constp.tile([P, 1], F32, name=f"b{i}")
                nc.sync.dma_start(bt[:], b_in[i][:])
                b_t.append(bt)
            blT_t = constp.tile([P, NC], F32)
            nc.sync.dma_start(blT_t[:], blT_in[:])

            idx_t = persist.tile([P, NB], I32)
            nc.sync.dma_start(idx_t[:], eidx_in[:])
            # one-hot scatter blocks live in SBUF for the whole kernel:
            # loaded once (overlapped with the startup barrier + layer 0)
            # instead of streamed from DRAM on every layer.
            oh_sb = persist.tile([P, NB * CW], MD)
            OHC = 8
            ohcols = NB * CW // OHC
            for j in range(OHC):
                nc.scalar.dma_start(
                    oh_sb[:, j * ohcols:(j + 1) * ohcols],
                    eoh_in[:, j * ohcols:(j + 1) * ohcols],
                )
            # layer-3 logits staging (log_softmax finished at the end)
            olog_t = persist.tile([CW, CH * NC], F32)
            sums_t = persist.tile([CW, CH], F32)

            tbl_shard = [
                None,
                dramp.tile([cfg.S_pad, P], MD, name="shard1"),
                dramp.tile([cfg.S_pad, P], MD, name="shard2"),
            ]
            # one Shared table per layer, each written by exactly one
            # one-shot AllGather (single-writer rule for Shared DRAM);
            # layer 1's table is the xf input itself
            tbl_full = [
                xf_in,
                dramp.tile([cfg.T_pad, P], MD, name="full1",
                           addr_space="Shared"),
                dramp.tile([cfg.T_pad, P], MD, name="full2",
                           addr_space="Shared"),
            ]

            def ag(l):
                nc.gpsimd.collective_compute(
                    "AllGather", mybir.AluOpType.bypass, replica_groups=rg,
                    ins=[tbl_shard[l][:, :].opt()],
                    outs=[tbl_full[l][:, :].opt()],
                )

            # message-passing layers
            for l in range(3):
                cur_psum = None
                pendA = []  # layer 1: (pair_id, msgS) awaiting stage A
                pend = []   # (pair_id, aT2) epilogues deferred so PE stays
                #             on message matmuls
                hb4_hold = [None]  # current 2-pair cast/write tile

                def emitA(pid, msgS):
                    # layer-1 stage A: deferred W1 (stationary, features
                    # stay on partitions) + ReLU; runs one pair ahead of
                    # stage B so the PE never waits on the ACT engine
                    h1p = opsump.tile(
                        [P, 2 * CW], F32, name="h1p", tag="o",
                    )
                    nc.tensor.matmul(
                        h1p[:], lhsT=W_t[0][:], rhs=msgS[:],
                        start=True, stop=True,
                    )
                    a1 = epip.tile([P, 2 * CW], F16, name="a1t",
                                   bufs=4)
                    nc.scalar.activation(
                        a1[:], h1p[:],
                        mybir.ActivationFunctionType.Relu,
                        bias=b_t[0][:, :1],
                    )
                    return (pid, a1)

                def emit(pid, aT2, l):
                    if True:
                        if l < 2:
                            hp2 = opsump.tile(
                                [2 * CW, P], F32, name="hp2", tag="o",
                            )
                            nc.tensor.matmul(
                                hp2[:], lhsT=aT2[:], rhs=W_t[l + 1][:],
                                start=True, stop=True,
                            )
                            # two pairs share one cast tile and one shard
                            # write: halves the write count (and thus the
                            # HBM write-completion semaphore waits that
                            # alias with gather completions on the shared
                            # DMA sem lanes); deep ring so the cast never
                            # waits on an old write either
                            if pid % 2 == 0:
                                hb4_hold[0] = epip.tile(
                                    [2 * CW, 2 * P], MD, name="hb", bufs=6,
                                )
                            hb4 = hb4_hold[0]
                            nc.vector.tensor_copy(
                                hb4[:, (pid % 2) * P:(pid % 2 + 1) * P],
                                hp2[:],
                            )
                            if pid % 2 == 1:
                                nc.sync.dma_start(
                                    tbl_shard[l + 1][
                                        (pid - 1) * 2 * CW:
                                        (pid + 1) * 2 * CW, :
                                    ].rearrange("(q p) f -> p q f",
                                                p=2 * CW),
                                    hb4[:, :].rearrange(
                                        "p (q f) -> p q f", q=2),
                                )
                        else:
                            for u in range(2):
                                cid = 2 * pid + u
                                lp = opsump.tile(
                                    [CW, NC], F32, name="lp", tag="o",
                                    padded_shape=[P, P],
                                )
                                nc.tensor.matmul(
                                    lp[:], lhsT=aT2[:, u * CW:(u + 1) * CW],
                                    rhs=Wl_t[:],
                                    start=True, stop=True,
                                )
                                nc.vector.tensor_tensor(
                                    olog_t[:, cid * NC:(cid + 1) * NC], lp[:],
                                    blT_t[:CW, :], mybir.AluOpType.add,
                                )
                            # inline softmax-denominator work on the idle
                            # ACT engine: logits are bounded (|z| < ~3) so
                            # exp needs no max shift in fp32
                            sl = olog_t[:, pid * 2 * NC:(pid + 1) * 2 * NC]
                            ex2 = epip.tile([CW, 2 * NC], F32, name="ex2",
                                            bufs=4)
                            nc.scalar.activation(
                                ex2[:], sl,
                                mybir.ActivationFunctionType.Exp,
                            )
                            nc.vector.reduce_sum(
                                sums_t[:, pid * 2:(pid + 1) * 2],
                                ex2[:, :].rearrange("p (c k) -> p c k", c=2),
                                axis=mybir.AxisListType.X,
                            )

                for g in range(NG):
                    gt = gatherp.tile([P, K * P], MD, name="gt")
                    nc.gpsimd.indirect_dma_start(
                        out=gt[:], out_offset=None,
                        in_=tbl_full[l][:],
                        in_offset=bass.IndirectOffsetOnAxis(
                            ap=idx_t[:, g * K:(g + 1) * K], axis=0
                        ),
                    )
                    for j in range(K):
                        b = g * K + j
                        cid, first, last = blocks[b]
                        even = (cid % 2) == 0
                        if first and even:
                            # one PSUM tile holds a chunk PAIR; the even
                            # chunk's start zeroes the whole 2KB region
                            cur_psum = mpsump.tile([P, 2 * CW], F32,
                                                   name="msg")
                        if DR is not None and j % 2 == 0:
                            # fp8 DoubleRow: contract blocks b and b+1 in
                            # one pass; psum[feat, w] +=
                            #   sum_i gt_i[e, feat].T @ onehot_i[e, w]
                            assert blocks[b + 1][0] == cid
                            last = blocks[b + 1][2]
                            nc.tensor.matmul(
                                cur_psum[:, (cid % 2) * CW:(cid % 2 + 1) * CW],
                                lhsT=gt[:, j * P:(j + 2) * P]
                                .rearrange("p (two m) -> p two m", two=2),
                                rhs=oh_sb[:, b * CW:(b + 2) * CW]
                                .rearrange("p (two n) -> p two n", two=2),
                                start=first and even,
                                stop=last and not even,
                                perf_mode=DR,
                            )
                        elif DR is not None:
                            continue
                        else:
                            # psum[feat, w] += gathered[e, feat].T
                            #                  @ onehot[e, w]
                            nc.tensor.matmul(
                                cur_psum[:, (cid % 2) * CW:(cid % 2 + 1) * CW],
                                lhsT=gt[:, ts(j, P)],
                                rhs=oh_sb[:, ts(b, CW)],
                                start=first and even,
                                stop=last and not even,
                            )
                        if not (last and not even):
                            continue
                        # pair epilogue: copy/bias(+ReLU) off the PE
                        # critical path, on ACT/DVE
                        aT2 = epip.tile([P, 2 * CW], F16, name="aT", bufs=8)
                        if l == 0:
                            # raw-x messages: W1 + ReLU are applied in the
                            # flush (after the deferred transform)
                            nc.vector.tensor_copy(aT2[:], cur_psum[:])
                        elif l == 1:
                            nc.scalar.activation(
                                aT2[:], cur_psum[:],
                                mybir.ActivationFunctionType.Relu,
                                bias=b_t[l][:, :1],
                            )
                        else:
                            nc.vector.tensor_scalar(
                                aT2[:], cur_psum[:], b_t[2][:, :1], None,
                                mybir.AluOpType.add,
                            )
                        # one-pair lags: each flush stage runs on a pair
                        # whose inputs are a full pair-period old, so the
                        # in-order PE queue never waits on ACT/DVE; the
                        # steady trickle also keeps epilogue psum slot
                        # reuse two pairs apart (no recycle stalls)
                        if l == 0:
                            pendA.append((cid // 2, aT2))
                            if len(pendA) > 1:
                                pend.append(emitA(*pendA.pop(0)))
                        else:
                            pend.append((cid // 2, aT2))
                        if len(pend) > 1:
                            emit(*pend.pop(0), l)
                for pa_ in pendA:
                    pend.append(emitA(*pa_))
                for pe_ in pend:
                    emit(*pe_, l)
                pend.clear()
                if l < 2:
                    ag(l + 1)

            # log_softmax tail: per-pair exp/sum already ran inside the
            # layer-3 flush, so only Ln + one subtract + output remain
            v3 = olog_t[:, :].rearrange("p (c k) -> p c k", c=CH)
            ln_t = persist.tile([CW, CH], F32)
            nc.scalar.activation(
                ln_t[:], sums_t[:], mybir.ActivationFunctionType.Ln
            )
            nc.vector.tensor_tensor(
                v3, v3,
                ln_t[:, :, None].broadcast_to((CW, CH, NC)),
                mybir.AluOpType.subtract,
            )
            nc.sync.dma_start(out_t[:], olog_t[:])
    nc.compile()
    return nc


def _make_in_maps(cfg, NB, per_core, x, W1, b1, W2, b2, W3, b3, Wl, bl):
    shared = {
        "W1": np.asarray(W1, np.float16),
        "W2": np.asarray(W2, np.float16),
        "W3": np.asarray(W3, np.float16),
        "Wl": np.asarray(Wl, np.float16),
        "b1": np.asarray(b1, np.float32).reshape(P, 1),
        "b2": np.asarray(b2, np.float32).reshape(P, 1),
        "b3": np.asarray(b3, np.float32).reshape(P, 1),
        "blT": np.broadcast_to(
            np.asarray(bl, np.float32)[None, :], (P, cfg.n_class)
        ).copy(),
    }
    # full node features in fp8, core-major padded rows (the same array
    # is uploaded to every core -- input staging, not float math)
    xf = np.zeros((cfg.T_pad, P), cfg.np_msg)
    for c in range(cfg.n_cores):
        xf[c * cfg.S_pad:c * cfg.S_pad + cfg.S] = np.asarray(
            x[c * cfg.S:(c + 1) * cfg.S]
        ).astype(cfg.np_msg)
    in_maps = []
    for c in range(cfg.n_cores):
        in_maps.append({
            "xf": xf,
            "eidx": per_core[c]["eidx"],
            "eoh": per_core[c]["eoh"],
            **shared,
        })
    return in_maps


def kernel(x, edge_index, W1, b1, W2, b2, W3, b3, Wl, bl):
    cfg = FULL
    x = np.asarray(x)
    edge_index = np.asarray(edge_index)
    NB, blocks, per_core = _preprocess(cfg, edge_index)
    nc = _build_program(cfg, NB, blocks)
    in_maps = _make_in_maps(
        cfg, NB, per_core, x, W1, b1, W2, b2, W3, b3, Wl, bl
    )
    res = run_bass_kernel_spmd(
        nc, in_maps, list(range(cfg.n_cores)),
        trace=bool(os.environ.get("GCN_TRACE")),
    )
    global LAST_RESULT
    LAST_RESULT = res
    out = np.empty((cfg.n_nodes, cfg.n_class), np.float32)
    for c in range(cfg.n_cores):
        r = np.asarray(res.results[c]["logits"])  # [CW, CH*NC]
        r = r.reshape(cfg.cw, cfg.CH, cfg.n_class).transpose(1, 0, 2)
        out[c * cfg.S:(c + 1) * cfg.S] = r.reshape(
            cfg.S_pad, cfg.n_class)[: cfg.S]
    return out
